# revision 12
# baseline (speedup 1.0000x reference)
"""Trainium2 Bass kernel for nn_AGCB_NoGCA (block non-local attention + conv/BN/ReLU).

Sharding: B*s*s = 8 blocks, one per NeuronCore. Each core runs attention on its
[256, 64, 64] block, cores exchange 1-pixel context borders via AllGather
(groups of 4 = same batch), then each core computes the 3x3 SAME conv on its
padded patch, BN folded into conv weights host-side, + residual ReLU.
"""

import os
import sys

import numpy as np

for _p in (
    "/root/.axon_site",
    "/root/.axon_site/_ro/trn_rl_repo",
    "/root/.axon_site/_ro/pypackages",
    "/opt/trn_rl_repo",
    "/opt/pypackages",
):
    if os.path.isdir(_p) and _p not in sys.path:
        sys.path.append(_p)

import ml_dtypes

B, C, H, W = 2, 256, 128, 128
S = 2
HB, WB = H // S, W // S  # 64, 64
L = HB * WB  # 4096
HID = 8
N_CORES = 8
BN_EPS = 1e-5

_NC_CACHE = {}


def _build_nc():
    import concourse.bass as bass
    import concourse.mybir as mybir
    import concourse.tile as tile
    from concourse import bacc

    f32 = mybir.dt.float32
    f32r = mybir.dt.float32r
    bf16 = mybir.dt.bfloat16
    AF = mybir.ActivationFunctionType
    OP = mybir.AluOpType

    nc = bacc.Bacc(
        "TRN2", target_bir_lowering=False, debug=False, num_devices=N_CORES)

    def dram_in(name, shape, dtype):
        return nc.dram_tensor(name, shape, dtype, kind="ExternalInput").ap()

    def dram_out(name, shape, dtype):
        return nc.dram_tensor(name, shape, dtype, kind="ExternalOutput").ap()

    x_ext = dram_in("x", [C, L], f32)
    qkwt_ext = dram_in("qkwt", [C, 2 * HID], bf16)
    qkb_ext = dram_in("qkb", [2 * HID, 1], f32)
    vwt_ext = dram_in("vwt", [C, C], bf16)
    cwt_ext = dram_in("cwt", [2, 128, 9 * 2 * 128], bf16)
    gnlvb_ext = dram_in("gnlvb", [C, 1], f32)
    bprime_ext = dram_in("bprime", [C, 1], f32)
    hmask_ext = dram_in("hmask", [128, 12], f32)
    gnlrow_ext = dram_in("gnlrow", [1, 128], bf16)
    ones_ext = dram_in("ones128", [128, 1], bf16)
    out_ext = dram_out("out", [C, L], f32)

    NLT = 8  # number of l tiles
    LT = L // NLT  # 512 = 8 rows of 64
    RPT = LT // WB  # rows per l-tile = 8
    NMC = L // 128  # 32 m-chunks

    with tile.TileContext(nc) as tc:
        with (
            tc.tile_pool(name="const", bufs=1) as constp,
            tc.tile_pool(name="xp", bufs=1) as xp,
            tc.tile_pool(name="qkp", bufs=1) as qkpool,
            tc.tile_pool(name="vtp", bufs=1) as vtpool,
            tc.tile_pool(name="padp", bufs=1) as padp,
            tc.tile_pool(name="dram", bufs=1, space="DRAM") as dram,
        ):
            # ---- load constants / inputs ----
            qkwt_sb, vwt_sb, cw_sb, gnlvb_sb, bprime_sb = [], [], [], [], []
            for icc in range(2):
                rows = slice(icc * 128, (icc + 1) * 128)
                t = constp.tile([128, 2 * HID], bf16, tag=f"qkwt{icc}", name=f"qkwt{icc}")
                nc.sync.dma_start(t[:, :], qkwt_ext[rows, :])
                qkwt_sb.append(t)
                t = constp.tile([128, C], bf16, tag=f"vwt{icc}", name=f"vwt{icc}")
                nc.sync.dma_start(t[:, :], vwt_ext[rows, :])
                vwt_sb.append(t)
                t = constp.tile([128, 9 * 2 * 128], bf16, tag=f"cw{icc}", name=f"cw{icc}")
                nc.sync.dma_start(t[:, :], cwt_ext[icc, :, :])
                cw_sb.append(t)
                t = constp.tile([128, 1], f32, tag=f"gnlvb{icc}", name=f"gnlvb{icc}")
                nc.sync.dma_start(t[:, :], gnlvb_ext[rows, :])
                gnlvb_sb.append(t)
                t = constp.tile([128, 1], f32, tag=f"bprime{icc}", name=f"bprime{icc}")
                nc.sync.dma_start(t[:, :], bprime_ext[rows, :])
                bprime_sb.append(t)
            qb_sb = constp.tile([HID, 1], f32, tag="qb")
            nc.sync.dma_start(qb_sb[:, :], qkb_ext[0:HID, :])
            kb_sb = constp.tile([HID, 1], f32, tag="kb")
            nc.sync.dma_start(kb_sb[:, :], qkb_ext[HID:2 * HID, :])
            hm = constp.tile([128, 12], f32, tag="hmask")
            nc.sync.dma_start(hm[:, :], hmask_ext[:, :])
            gnlrow_sb = constp.tile([1, 128], bf16, tag="gnlrow")
            nc.sync.dma_start(gnlrow_sb[:, :], gnlrow_ext[:, :])
            ones_sb = constp.tile([128, 1], bf16, tag="ones128")
            nc.sync.dma_start(ones_sb[:, :], ones_ext[:, :])

            X = []  # 2 x [128, 64, 64] f32
            for cc in range(2):
                t = xp.tile([128, HB, WB], f32, tag=f"x{cc}", name=f"x{cc}")
                nc.sync.dma_start(t[:, :, :], x_ext[cc * 128:(cc + 1) * 128, :].rearrange("c (h w) -> c h w", h=HB))
                X.append(t)
            Xb = []  # bf16 copies for matmul operands
            for cc in range(2):
                t = xp.tile([128, HB, WB], bf16, tag=f"xb{cc}", name=f"xb{cc}")
                if cc == 0:
                    nc.vector.tensor_copy(t[:, :, :], X[cc][:, :, :])
                else:
                    nc.scalar.copy(t[:, :, :], X[cc][:, :, :])
                Xb.append(t)

            q_sb = qkpool.tile([HID, L], bf16, tag="q")
            k_sb = qkpool.tile([HID, L], bf16, tag="k")
            vt = vtpool.tile([128, NMC, C], bf16, tag="vt")
            pad = []
            for cc in range(2):
                pad.append(padp.tile([128, HB + 2, WB + 2], bf16, tag=f"pad{cc}", name=f"pad{cc}"))

            # ---- phase 1: q/k = (qkw @ x) + b, bf16 ----
            with tc.tile_pool(name="ps_qk", bufs=2, space="PSUM") as ps_qk:
                NPC = 8  # pieces
                PL = L // NPC  # 512
                for p in range(NPC):
                    qps = ps_qk.tile([HID, PL], f32, tag="qps", name=f"qps{p}")
                    kps = ps_qk.tile([HID, PL], f32, tag="kps", name=f"kps{p}")
                    for icc in range(2):
                        xs = Xb[icc][:, :, :].rearrange("c h w -> c (h w)")[:, p * PL:(p + 1) * PL]
                        nc.tensor.matmul(
                            qps[:, :], lhsT=qkwt_sb[icc][:, 0:HID], rhs=xs,
                            start=(icc == 0), stop=(icc == 1))
                        nc.tensor.matmul(
                            kps[:, :], lhsT=qkwt_sb[icc][:, HID:2 * HID], rhs=xs,
                            start=(icc == 0), stop=(icc == 1))
                    nc.vector.tensor_scalar_add(
                        q_sb[:, p * PL:(p + 1) * PL], qps[:, :], qb_sb[:, 0:1])
                    nc.vector.tensor_scalar_add(
                        k_sb[:, p * PL:(p + 1) * PL], kps[:, :], kb_sb[:, 0:1])

                # ---- phase 2: vT[m, c] = x^T @ vw^T  (no bias; vb folded later) ----
                with tc.tile_pool(name="ps_vt", bufs=3, space="PSUM") as ps_vt:
                    for mc in range(NMC):
                        vtps = ps_vt.tile([128, C], f32, tag="vtps")
                        for icc in range(2):
                            nc.tensor.matmul(
                                vtps[:, :],
                                lhsT=Xb[icc][:, :, :].rearrange("c h w -> c (h w)")[:, mc * 128:(mc + 1) * 128],
                                rhs=vwt_sb[icc][:, :],
                                start=(icc == 0),
                                stop=(icc == 1),
                            )
                        nc.vector.tensor_copy(vt[:, mc, :], vtps[:, :])

            # ---- phase 3: attention per l-tile ----
            with (
                tc.tile_pool(name="ps_e", bufs=2, space="PSUM") as ps_e,
                tc.tile_pool(name="ps_o", bufs=3, space="PSUM") as ps_o,
                tc.tile_pool(name="ps_s", bufs=1, space="PSUM") as ps_s,
                tc.tile_pool(name="ps_r", bufs=1, space="PSUM") as ps_r,
                tc.tile_pool(name="etp", bufs=3) as etp,
                tc.tile_pool(name="sump", bufs=2) as sump,
                tc.tile_pool(name="rp", bufs=2) as rp,
                tc.tile_pool(name="tmpp", bufs=3) as tmpp,
            ):
                for lt in range(NLT):
                    q_lt = q_sb[:, lt * LT:(lt + 1) * LT]
                    o_ps = [ps_o.tile([128, LT], f32, tag="ops", name=f"ops{lt}_{i}") for i in range(2)]
                    sumE = sump.tile([128, LT], bf16, tag="sumE")
                    for mc in range(NMC):
                        e_ps = ps_e.tile([128, LT], f32, tag="eps")
                        nc.tensor.matmul(
                            e_ps[:, :],
                            lhsT=k_sb[:, mc * 128:(mc + 1) * 128],
                            rhs=q_lt,
                            start=True, stop=True,
                        )
                        et = etp.tile([128, LT], bf16, tag="et")
                        nc.scalar.activation(et[:, :], e_ps[:, :], AF.Exp)
                        for cc in range(2):
                            nc.tensor.matmul(
                                o_ps[cc][:, :],
                                lhsT=vt[:, mc, cc * 128:(cc + 1) * 128],
                                rhs=et[:, :],
                                start=(mc == 0), stop=(mc == NMC - 1),
                                skip_group_check=True,
                            )
                        if mc == 0:
                            nc.vector.tensor_copy(sumE[:, :], et[:, :])
                        else:
                            nc.vector.tensor_add(sumE[:, :], sumE[:, :], et[:, :])
                    # S = ones^T @ sumE ; r128 = gamma_nl / S broadcast to 128 parts
                    s_ps = ps_s.tile([1, LT], f32, tag="sps")
                    nc.tensor.matmul(s_ps[:, :], lhsT=ones_sb[:, :], rhs=sumE[:, :], start=True, stop=True)
                    r_f = rp.tile([1, LT], f32, tag="rf")
                    nc.vector.reciprocal(r_f[:, :], s_ps[:, :])
                    r_b = rp.tile([1, LT], bf16, tag="rb")
                    nc.vector.tensor_copy(r_b[:, :], r_f[:, :])
                    r_ps = ps_r.tile([128, LT], f32, tag="rps")
                    nc.tensor.matmul(r_ps[:, :], lhsT=gnlrow_sb[:, :], rhs=r_b[:, :], start=True, stop=True)
                    r128 = rp.tile([128, LT], f32, tag="r128")
                    nc.vector.tensor_copy(r128[:, :], r_ps[:, :])
                    # ctx = O * r128 + gnl*vb + x  -> written into padded interior
                    for cc in range(2):
                        t = tmpp.tile([128, LT], f32, tag="ctx_t")
                        nc.vector.tensor_mul(t[:, :], o_ps[cc][:, :], r128[:, :])
                        nc.vector.scalar_tensor_tensor(
                            out=pad[cc][:, 1 + lt * RPT:1 + (lt + 1) * RPT, 1:1 + WB],
                            in0=t[:, :],
                            scalar=gnlvb_sb[cc][:, 0:1],
                            in1=X[cc][:, lt * RPT:(lt + 1) * RPT, :],
                            op0=OP.add,
                            op1=OP.add,
                        )

            # ---- phase 4: border exchange (AllGather within groups of 4) ----
            cc_in = dram.tile([C, 256], bf16, tag="cc_in")
            cc_out = dram.tile([4 * C, 256], bf16, tag="cc_out")
            for cc in range(2):
                rows = slice(cc * 128, (cc + 1) * 128)
                nc.sync.dma_start(cc_in[rows, 0:64], pad[cc][:, 1, 1:65])
                nc.sync.dma_start(cc_in[rows, 64:128], pad[cc][:, 64, 1:65])
                nc.sync.dma_start(cc_in[rows, 128:192], pad[cc][:, 1:65, 1])
                nc.sync.dma_start(cc_in[rows, 192:256], pad[cc][:, 1:65, 64])
            nc.gpsimd.collective_compute(
                "AllGather",
                mybir.AluOpType.bypass,
                replica_groups=[[0, 1, 2, 3], [4, 5, 6, 7]],
                ins=[cc_in[:, :].opt()],
                outs=[cc_out[:, :].opt()],
            )
            with tc.tile_pool(name="gp", bufs=16) as gp, tc.tile_pool(name="hwt", bufs=4) as hwt:
                for cc in range(2):
                    def gload(rank, col0, tag):
                        t = gp.tile([128, 64], bf16, tag=f"g_{tag}_{cc}", name=f"g_{tag}_{cc}")
                        nc.sync.dma_start(
                            t[:, :], cc_out[rank * C + cc * 128: rank * C + (cc + 1) * 128, col0:col0 + 64])
                        return t
                    g_top = [gload(0, 64, "t0"), gload(1, 64, "t1")]   # ranks 0,1 bottom rows
                    g_bot = [gload(2, 0, "b0"), gload(3, 0, "b1")]     # ranks 2,3 top rows
                    g_lef = [gload(0, 192, "l0"), gload(2, 192, "l1")]  # ranks 0,2 right cols
                    g_rig = [gload(1, 128, "r0"), gload(3, 128, "r1")]  # ranks 1,3 left cols
                    sides = [
                        (g_top, 0, 1, pad[cc][:, 0, 1:65]),
                        (g_bot, 2, 3, pad[cc][:, 65, 1:65]),
                        (g_lef, 4, 5, pad[cc][:, 1:65, 0]),
                        (g_rig, 6, 7, pad[cc][:, 1:65, 65]),
                    ]
                    for (gt, i0, i1, dst) in sides:
                        w = hwt.tile([128, 64], bf16, tag=f"hw_{cc}")
                        nc.vector.tensor_scalar_mul(w[:, :], gt[0][:, :], hm[:, i0:i0 + 1])
                        nc.vector.scalar_tensor_tensor(
                            out=dst, in0=gt[1][:, :], scalar=hm[:, i1:i1 + 1], in1=w[:, :],
                            op0=OP.mult, op1=OP.add)
                    # corners: TL, TR, BL, BR
                    nc.vector.tensor_scalar_mul(pad[cc][:, 0, 0:1], g_top[0][:, 63:64], hm[:, 8:9])
                    nc.vector.tensor_scalar_mul(pad[cc][:, 0, 65:66], g_top[1][:, 0:1], hm[:, 9:10])
                    nc.vector.tensor_scalar_mul(pad[cc][:, 65, 0:1], g_bot[0][:, 63:64], hm[:, 10:11])
                    nc.vector.tensor_scalar_mul(pad[cc][:, 65, 65:66], g_bot[1][:, 0:1], hm[:, 11:12])

            # ---- phase 5: conv 3x3 (BN folded) + residual relu ----
            with (
                tc.tile_pool(name="ps_c", bufs=4, space="PSUM") as ps_c,
                tc.tile_pool(name="outp", bufs=4) as outp,
            ):
                for occ in range(2):
                    for rb_i in range(NLT):
                        cps = ps_c.tile([128, RPT * WB], f32, tag="cps")
                        first = True
                        for icc in range(2):
                            for dy in range(3):
                                for dx in range(3):
                                    tap = dy * 3 + dx
                                    nc.tensor.matmul(
                                        cps[:, :],
                                        lhsT=cw_sb[icc][:, (tap * 2 + occ) * 128:(tap * 2 + occ + 1) * 128],
                                        rhs=pad[icc][:, rb_i * RPT + dy: rb_i * RPT + dy + RPT, dx:dx + WB],
                                        start=first,
                                        stop=(icc == 1 and tap == 8),
                                    )
                                    first = False
                        u = outp.tile([128, RPT * WB], f32, tag="u")
                        nc.vector.scalar_tensor_tensor(
                            out=u[:, :], in0=cps[:, :],
                            scalar=bprime_sb[occ][:, 0:1],
                            in1=X[occ][:, rb_i * RPT:(rb_i + 1) * RPT, :],
                            op0=OP.add, op1=OP.add)
                        o = outp.tile([128, RPT * WB], f32, tag="o")
                        nc.vector.tensor_scalar_max(o[:, :], u[:, :], 0.0)
                        nc.sync.dma_start(
                            out_ext[occ * 128:(occ + 1) * 128, rb_i * LT:(rb_i + 1) * LT], o[:, :])

    nc.compile()
    return nc


def _get_nc():
    if "nc" not in _NC_CACHE:
        _NC_CACHE["nc"] = _build_nc()
    return _NC_CACHE["nc"]


def _prep_inputs(x, qw, qb, kw, kb, vw, vb, gamma_nl, conv_w, conv_b,
                 bn_w, bn_b, bn_mean, bn_var, gamma):
    x = np.asarray(x, np.float32)
    gamma = float(np.asarray(gamma).reshape(-1)[0])
    gamma_nl = float(np.asarray(gamma_nl).reshape(-1)[0])

    blocks = (
        x.reshape(B, C, S, HB, S, WB)
        .transpose(0, 2, 4, 1, 3, 5)
        .reshape(N_CORES, C, L)
    )

    qkwt = np.ascontiguousarray(np.concatenate([qw, kw], 0).T).astype(ml_dtypes.bfloat16)  # [256,16]
    qkb = np.concatenate([qb, kb], 0).reshape(2 * HID, 1).astype(np.float32)
    vwt = np.ascontiguousarray(np.asarray(vw, np.float32).T).astype(ml_dtypes.bfloat16)  # [256,256]

    inv = 1.0 / np.sqrt(np.asarray(bn_var, np.float64) + BN_EPS)
    A = (gamma * np.asarray(bn_w, np.float64) * inv).astype(np.float32)  # [256]
    Bp = (gamma * ((np.asarray(conv_b, np.float64) - np.asarray(bn_mean, np.float64))
                   * np.asarray(bn_w, np.float64) * inv + np.asarray(bn_b, np.float64))).astype(np.float32)
    Wp = (np.asarray(conv_w, np.float32) * A[:, None, None, None]).astype(np.float32)  # [o,i,dy,dx]
    # layout [icc, ic128, tap, occ, oc128] -> [2, 128, 9*2*128]
    cwt = np.ascontiguousarray(
        Wp.transpose(1, 2, 3, 0).reshape(2, 128, 3, 3, 2, 128).reshape(2, 128, 9 * 2 * 128)
    ).astype(ml_dtypes.bfloat16)

    gnlvb = (gamma_nl * np.asarray(vb, np.float32)).reshape(C, 1).astype(np.float32)
    bprime = Bp.reshape(C, 1)
    gnlrow = np.full((1, 128), gamma_nl, ml_dtypes.bfloat16)
    ones128 = np.ones((128, 1), ml_dtypes.bfloat16)

    in_maps = []
    for core in range(N_CORES):
        r = core % 4
        m = np.zeros(12, np.float32)
        m[0] = 1.0 if r == 2 else 0.0   # top halo from G0.bottom
        m[1] = 1.0 if r == 3 else 0.0   # top halo from G1.bottom
        m[2] = 1.0 if r == 0 else 0.0   # bottom halo from G2.top
        m[3] = 1.0 if r == 1 else 0.0   # bottom halo from G3.top
        m[4] = 1.0 if r == 1 else 0.0   # left halo from G0.right
        m[5] = 1.0 if r == 3 else 0.0   # left halo from G2.right
        m[6] = 1.0 if r == 0 else 0.0   # right halo from G1.left
        m[7] = 1.0 if r == 2 else 0.0   # right halo from G3.left
        m[8] = 1.0 if r == 3 else 0.0   # TL corner from g_top0[:,63]
        m[9] = 1.0 if r == 2 else 0.0   # TR corner from g_top1[:,0]
        m[10] = 1.0 if r == 1 else 0.0  # BL corner from g_bot0[:,63]
        m[11] = 1.0 if r == 0 else 0.0  # BR corner from g_bot1[:,0]
        hmask = np.tile(m[None, :], (128, 1)).astype(np.float32)
        in_maps.append({
            "x": np.ascontiguousarray(blocks[core]),
            "qkwt": qkwt, "qkb": qkb, "vwt": vwt, "cwt": cwt,
            "gnlvb": gnlvb, "bprime": bprime, "hmask": hmask,
            "gnlrow": gnlrow, "ones128": ones128,
        })
    return in_maps


def _assemble(outs):
    ob = np.stack([np.asarray(o, np.float32) for o in outs], 0)  # [8, 256, 4096]
    return (
        ob.reshape(B, S, S, C, HB, WB)
        .transpose(0, 3, 1, 4, 2, 5)
        .reshape(B, C, H, W)
    )


def run_on_hw(in_maps, trace=False, trace_kwargs=None):
    from concourse import bass_utils
    nc = _get_nc()
    res = bass_utils.run_bass_kernel_spmd(
        nc, in_maps, core_ids=list(range(N_CORES)), trace=trace,
        **(trace_kwargs or {}))
    return res


def kernel(**inputs):
    in_maps = _prep_inputs(**inputs)
    res = run_on_hw(in_maps)
    outs = [r["out"] for r in res.results]
    return _assemble(outs)


if __name__ == "__main__":
    pass


# revision 13
# speedup vs baseline: 1.9306x; 1.9306x over previous
"""Trainium2 Bass kernel for nn_AGCB_NoGCA (block non-local attention + conv/BN/ReLU).

Sharding: B*s*s = 8 blocks, one per NeuronCore. Each core runs attention on its
[256, 64, 64] block, cores exchange 1-pixel context borders via AllGather
(groups of 4 = same batch), then each core computes the 3x3 SAME conv on its
padded patch, BN folded into conv weights host-side, + residual ReLU.
"""

import os
import sys

import numpy as np

for _p in (
    "/root/.axon_site",
    "/root/.axon_site/_ro/trn_rl_repo",
    "/root/.axon_site/_ro/pypackages",
    "/opt/trn_rl_repo",
    "/opt/pypackages",
):
    if os.path.isdir(_p) and _p not in sys.path:
        sys.path.append(_p)

import ml_dtypes

B, C, H, W = 2, 256, 128, 128
S = 2
HB, WB = H // S, W // S  # 64, 64
L = HB * WB  # 4096
HID = 8
N_CORES = 8
BN_EPS = 1e-5

_NC_CACHE = {}


def _build_nc():
    import concourse.bass as bass
    import concourse.mybir as mybir
    import concourse.tile as tile
    from concourse import bacc

    f32 = mybir.dt.float32
    f32r = mybir.dt.float32r
    bf16 = mybir.dt.bfloat16
    AF = mybir.ActivationFunctionType
    OP = mybir.AluOpType

    nc = bacc.Bacc(
        "TRN2", target_bir_lowering=False, debug=False, num_devices=N_CORES)

    def dram_in(name, shape, dtype):
        return nc.dram_tensor(name, shape, dtype, kind="ExternalInput").ap()

    def dram_out(name, shape, dtype):
        return nc.dram_tensor(name, shape, dtype, kind="ExternalOutput").ap()

    x_ext = dram_in("x", [C, L], f32)
    qkwt_ext = dram_in("qkwt", [C, 2 * HID], bf16)
    qkb_ext = dram_in("qkb", [2 * HID, 1], f32)
    vwt_ext = dram_in("vwt", [C, C], bf16)
    cwt_ext = dram_in("cwt", [2, 128, 9 * 2 * 128], bf16)
    gnlvb_ext = dram_in("gnlvb", [C, 1], f32)
    bprime_ext = dram_in("bprime", [C, 1], f32)
    hmask_ext = dram_in("hmask", [128, 12], f32)
    gnlrow_ext = dram_in("gnlrow", [1, 128], bf16)
    ones_ext = dram_in("ones128", [128, 1], bf16)
    out_ext = dram_out("out", [C, L], f32)

    NLT = 8  # number of l tiles
    LT = L // NLT  # 512 = 8 rows of 64
    RPT = LT // WB  # rows per l-tile = 8
    NMC = L // 128  # 32 m-chunks

    with tile.TileContext(nc) as tc:
        with (
            tc.tile_pool(name="const", bufs=1) as constp,
            tc.tile_pool(name="xp", bufs=1) as xp,
            tc.tile_pool(name="qkp", bufs=1) as qkpool,
            tc.tile_pool(name="vtp", bufs=1) as vtpool,
            tc.tile_pool(name="padp", bufs=1) as padp,
            tc.tile_pool(name="dram", bufs=1, space="DRAM") as dram,
        ):
            # ---- load constants / inputs ----
            qkwt_sb, vwt_sb, cw_sb, gnlvb_sb, bprime_sb = [], [], [], [], []
            for icc in range(2):
                rows = slice(icc * 128, (icc + 1) * 128)
                t = constp.tile([128, 2 * HID], bf16, tag=f"qkwt{icc}", name=f"qkwt{icc}")
                nc.sync.dma_start(t[:, :], qkwt_ext[rows, :])
                qkwt_sb.append(t)
                t = constp.tile([128, C], bf16, tag=f"vwt{icc}", name=f"vwt{icc}")
                nc.sync.dma_start(t[:, :], vwt_ext[rows, :])
                vwt_sb.append(t)
                t = constp.tile([128, 9 * 2 * 128], bf16, tag=f"cw{icc}", name=f"cw{icc}")
                nc.sync.dma_start(t[:, :], cwt_ext[icc, :, :])
                cw_sb.append(t)
                t = constp.tile([128, 1], f32, tag=f"gnlvb{icc}", name=f"gnlvb{icc}")
                nc.sync.dma_start(t[:, :], gnlvb_ext[rows, :])
                gnlvb_sb.append(t)
                t = constp.tile([128, 1], f32, tag=f"bprime{icc}", name=f"bprime{icc}")
                nc.sync.dma_start(t[:, :], bprime_ext[rows, :])
                bprime_sb.append(t)
            qb_sb = constp.tile([HID, 1], f32, tag="qb")
            nc.sync.dma_start(qb_sb[:, :], qkb_ext[0:HID, :])
            kb_sb = constp.tile([HID, 1], f32, tag="kb")
            nc.sync.dma_start(kb_sb[:, :], qkb_ext[HID:2 * HID, :])
            hm = constp.tile([128, 12], f32, tag="hmask")
            nc.sync.dma_start(hm[:, :], hmask_ext[:, :])
            gnlrow_sb = constp.tile([1, 128], bf16, tag="gnlrow")
            nc.sync.dma_start(gnlrow_sb[:, :], gnlrow_ext[:, :])
            ones_sb = constp.tile([128, 1], bf16, tag="ones128")
            nc.sync.dma_start(ones_sb[:, :], ones_ext[:, :])

            X = []  # 2 x [128, 64, 64] f32
            for cc in range(2):
                t = xp.tile([128, HB, WB], f32, tag=f"x{cc}", name=f"x{cc}")
                nc.sync.dma_start(t[:, :, :], x_ext[cc * 128:(cc + 1) * 128, :].rearrange("c (h w) -> c h w", h=HB))
                X.append(t)
            Xb = []  # bf16 copies for matmul operands
            for cc in range(2):
                t = xp.tile([128, HB, WB], bf16, tag=f"xb{cc}", name=f"xb{cc}")
                if cc == 0:
                    nc.vector.tensor_copy(t[:, :, :], X[cc][:, :, :])
                else:
                    nc.scalar.copy(t[:, :, :], X[cc][:, :, :])
                Xb.append(t)

            q_sb = qkpool.tile([HID, L], bf16, tag="q")
            k_sb = qkpool.tile([HID, L], bf16, tag="k")
            vt = vtpool.tile([128, NMC, C], bf16, tag="vt")
            pad = []
            for cc in range(2):
                pad.append(padp.tile([128, HB + 2, WB + 2], bf16, tag=f"pad{cc}", name=f"pad{cc}"))

            # ---- phase 1: q/k = (qkw @ x) + b, bf16 ----
            with tc.tile_pool(name="ps_qk", bufs=2, space="PSUM") as ps_qk:
                NPC = 8  # pieces
                PL = L // NPC  # 512
                for p in range(NPC):
                    qps = ps_qk.tile([HID, PL], f32, tag="qps", name=f"qps{p}")
                    kps = ps_qk.tile([HID, PL], f32, tag="kps", name=f"kps{p}")
                    for icc in range(2):
                        xs = Xb[icc][:, :, :].rearrange("c h w -> c (h w)")[:, p * PL:(p + 1) * PL]
                        nc.tensor.matmul(
                            qps[:, :], lhsT=qkwt_sb[icc][:, 0:HID], rhs=xs,
                            start=(icc == 0), stop=(icc == 1))
                        nc.tensor.matmul(
                            kps[:, :], lhsT=qkwt_sb[icc][:, HID:2 * HID], rhs=xs,
                            start=(icc == 0), stop=(icc == 1))
                    nc.vector.tensor_scalar_add(
                        q_sb[:, p * PL:(p + 1) * PL], qps[:, :], qb_sb[:, 0:1])
                    nc.vector.tensor_scalar_add(
                        k_sb[:, p * PL:(p + 1) * PL], kps[:, :], kb_sb[:, 0:1])

                # ---- phase 2: vT[m, c] = x^T @ vw^T  (no bias; vb folded later) ----
                with tc.tile_pool(name="ps_vt", bufs=3, space="PSUM") as ps_vt:
                    for mc in range(NMC):
                        vtps = ps_vt.tile([128, C], f32, tag="vtps")
                        for icc in range(2):
                            nc.tensor.matmul(
                                vtps[:, :],
                                lhsT=Xb[icc][:, :, :].rearrange("c h w -> c (h w)")[:, mc * 128:(mc + 1) * 128],
                                rhs=vwt_sb[icc][:, :],
                                start=(icc == 0),
                                stop=(icc == 1),
                            )
                        nc.vector.tensor_copy(vt[:, mc, :], vtps[:, :])

            # ---- phase 3: attention per l-tile ----
            with (
                tc.tile_pool(name="ps_e", bufs=2, space="PSUM") as ps_e,
                tc.tile_pool(name="ps_o", bufs=3, space="PSUM") as ps_o,
                tc.tile_pool(name="ps_s", bufs=1, space="PSUM") as ps_s,
                tc.tile_pool(name="ps_r", bufs=1, space="PSUM") as ps_r,
                tc.tile_pool(name="etp", bufs=3) as etp,
                tc.tile_pool(name="sump", bufs=2) as sump,
                tc.tile_pool(name="rp", bufs=2) as rp,
                tc.tile_pool(name="tmpp", bufs=3) as tmpp,
            ):
                for lt in range(NLT):
                    q_lt = q_sb[:, lt * LT:(lt + 1) * LT]
                    o_ps = [ps_o.tile([128, LT], f32, tag="ops", name=f"ops{lt}_{i}") for i in range(2)]
                    sumE = sump.tile([128, LT], bf16, tag="sumE")
                    for mc in range(NMC):
                        e_ps = ps_e.tile([128, LT], f32, tag="eps")
                        nc.tensor.matmul(
                            e_ps[:, :],
                            lhsT=k_sb[:, mc * 128:(mc + 1) * 128],
                            rhs=q_lt,
                            start=True, stop=True,
                        )
                        et = etp.tile([128, LT], bf16, tag="et")
                        nc.scalar.activation(et[:, :], e_ps[:, :], AF.Exp)
                        for cc in range(2):
                            nc.tensor.matmul(
                                o_ps[cc][:, :],
                                lhsT=vt[:, mc, cc * 128:(cc + 1) * 128],
                                rhs=et[:, :],
                                start=(mc == 0), stop=(mc == NMC - 1),
                                skip_group_check=True,
                            )
                        if mc == 0:
                            nc.vector.tensor_copy(sumE[:, :], et[:, :])
                        else:
                            nc.vector.tensor_add(sumE[:, :], sumE[:, :], et[:, :])
                    # S = ones^T @ sumE ; r128 = gamma_nl / S broadcast to 128 parts
                    s_ps = ps_s.tile([1, LT], f32, tag="sps")
                    nc.tensor.matmul(s_ps[:, :], lhsT=ones_sb[:, :], rhs=sumE[:, :], start=True, stop=True)
                    r_f = rp.tile([1, LT], f32, tag="rf")
                    nc.vector.reciprocal(r_f[:, :], s_ps[:, :])
                    r_b = rp.tile([1, LT], bf16, tag="rb")
                    nc.vector.tensor_copy(r_b[:, :], r_f[:, :])
                    r_ps = ps_r.tile([128, LT], f32, tag="rps")
                    nc.tensor.matmul(r_ps[:, :], lhsT=gnlrow_sb[:, :], rhs=r_b[:, :], start=True, stop=True)
                    r128 = rp.tile([128, LT], f32, tag="r128")
                    nc.vector.tensor_copy(r128[:, :], r_ps[:, :])
                    # ctx = O * r128 + gnl*vb + x  -> written into padded interior
                    for cc in range(2):
                        t = tmpp.tile([128, LT], f32, tag="ctx_t")
                        nc.vector.tensor_mul(t[:, :], o_ps[cc][:, :], r128[:, :])
                        nc.vector.scalar_tensor_tensor(
                            out=pad[cc][:, 1 + lt * RPT:1 + (lt + 1) * RPT, 1:1 + WB],
                            in0=t[:, :],
                            scalar=gnlvb_sb[cc][:, 0:1],
                            in1=X[cc][:, lt * RPT:(lt + 1) * RPT, :],
                            op0=OP.add,
                            op1=OP.add,
                        )

            # ---- phase 4: border exchange (AllGather within groups of 4) ----
            # Stage the 4 borders contiguously in SBUF first so every DMA is
            # one big per-partition run (the strided column extract would
            # otherwise explode into elementwise DMA descriptors).
            cc_in = dram.tile([128, 512], bf16, tag="cc_in")
            cc_out = dram.tile([4 * 128, 512], bf16, tag="cc_out")
            with tc.tile_pool(name="stgp", bufs=2) as stgp:
                for cc in range(2):
                    stg = stgp.tile([128, 4, 64], bf16, tag="stg", name=f"stg{cc}")
                    nc.vector.tensor_copy(stg[:, 0, :], pad[cc][:, 1, 1:65])
                    nc.vector.tensor_copy(stg[:, 1, :], pad[cc][:, 64, 1:65])
                    nc.vector.tensor_copy(stg[:, 2, :], pad[cc][:, 1:65, 1])
                    nc.vector.tensor_copy(stg[:, 3, :], pad[cc][:, 1:65, 64])
                    nc.sync.dma_start(cc_in[:, cc * 256:(cc + 1) * 256], stg[:, :, :])
            nc.gpsimd.collective_compute(
                "AllGather",
                mybir.AluOpType.bypass,
                replica_groups=[[0, 1, 2, 3], [4, 5, 6, 7]],
                ins=[cc_in[:, :].opt()],
                outs=[cc_out[:, :].opt()],
            )
            with tc.tile_pool(name="gp", bufs=16) as gp, tc.tile_pool(name="hwt", bufs=4) as hwt:
                for cc in range(2):
                    gin = []
                    for rank in range(4):
                        t = gp.tile([128, 4, 64], bf16, tag=f"gin{rank}_{cc}", name=f"gin{rank}_{cc}")
                        nc.sync.dma_start(
                            t[:, :, :],
                            cc_out[rank * 128:(rank + 1) * 128, cc * 256:(cc + 1) * 256])
                        gin.append(t)
                    g_top = [gin[0][:, 1, :], gin[1][:, 1, :]]  # ranks 0,1 bottom rows
                    g_bot = [gin[2][:, 0, :], gin[3][:, 0, :]]  # ranks 2,3 top rows
                    g_lef = [gin[0][:, 3, :], gin[2][:, 3, :]]  # ranks 0,2 right cols
                    g_rig = [gin[1][:, 2, :], gin[3][:, 2, :]]  # ranks 1,3 left cols
                    sides = [
                        (g_top, 0, 1, pad[cc][:, 0, 1:65]),
                        (g_bot, 2, 3, pad[cc][:, 65, 1:65]),
                        (g_lef, 4, 5, pad[cc][:, 1:65, 0]),
                        (g_rig, 6, 7, pad[cc][:, 1:65, 65]),
                    ]
                    for (gt, i0, i1, dst) in sides:
                        w = hwt.tile([128, 64], bf16, tag=f"hw_{cc}")
                        nc.vector.tensor_scalar_mul(w[:, :], gt[0], hm[:, i0:i0 + 1])
                        nc.vector.scalar_tensor_tensor(
                            out=dst, in0=gt[1], scalar=hm[:, i1:i1 + 1], in1=w[:, :],
                            op0=OP.mult, op1=OP.add)
                    # corners: TL, TR, BL, BR
                    nc.vector.tensor_scalar_mul(pad[cc][:, 0, 0:1], g_top[0][:, 63:64], hm[:, 8:9])
                    nc.vector.tensor_scalar_mul(pad[cc][:, 0, 65:66], g_top[1][:, 0:1], hm[:, 9:10])
                    nc.vector.tensor_scalar_mul(pad[cc][:, 65, 0:1], g_bot[0][:, 63:64], hm[:, 10:11])
                    nc.vector.tensor_scalar_mul(pad[cc][:, 65, 65:66], g_bot[1][:, 0:1], hm[:, 11:12])

            # ---- phase 5: conv 3x3 (BN folded) + residual relu ----
            with (
                tc.tile_pool(name="ps_c", bufs=4, space="PSUM") as ps_c,
                tc.tile_pool(name="outp", bufs=4) as outp,
            ):
                for occ in range(2):
                    for rb_i in range(NLT):
                        cps = ps_c.tile([128, RPT * WB], f32, tag="cps")
                        first = True
                        for icc in range(2):
                            for dy in range(3):
                                for dx in range(3):
                                    tap = dy * 3 + dx
                                    nc.tensor.matmul(
                                        cps[:, :],
                                        lhsT=cw_sb[icc][:, (tap * 2 + occ) * 128:(tap * 2 + occ + 1) * 128],
                                        rhs=pad[icc][:, rb_i * RPT + dy: rb_i * RPT + dy + RPT, dx:dx + WB],
                                        start=first,
                                        stop=(icc == 1 and tap == 8),
                                    )
                                    first = False
                        u = outp.tile([128, RPT * WB], f32, tag="u")
                        nc.vector.scalar_tensor_tensor(
                            out=u[:, :], in0=cps[:, :],
                            scalar=bprime_sb[occ][:, 0:1],
                            in1=X[occ][:, rb_i * RPT:(rb_i + 1) * RPT, :],
                            op0=OP.add, op1=OP.add)
                        o = outp.tile([128, RPT * WB], f32, tag="o")
                        nc.vector.tensor_scalar_max(o[:, :], u[:, :], 0.0)
                        nc.sync.dma_start(
                            out_ext[occ * 128:(occ + 1) * 128, rb_i * LT:(rb_i + 1) * LT], o[:, :])

    nc.compile()
    return nc


def _get_nc():
    if "nc" not in _NC_CACHE:
        _NC_CACHE["nc"] = _build_nc()
    return _NC_CACHE["nc"]


def _prep_inputs(x, qw, qb, kw, kb, vw, vb, gamma_nl, conv_w, conv_b,
                 bn_w, bn_b, bn_mean, bn_var, gamma):
    x = np.asarray(x, np.float32)
    gamma = float(np.asarray(gamma).reshape(-1)[0])
    gamma_nl = float(np.asarray(gamma_nl).reshape(-1)[0])

    blocks = (
        x.reshape(B, C, S, HB, S, WB)
        .transpose(0, 2, 4, 1, 3, 5)
        .reshape(N_CORES, C, L)
    )

    qkwt = np.ascontiguousarray(np.concatenate([qw, kw], 0).T).astype(ml_dtypes.bfloat16)  # [256,16]
    qkb = np.concatenate([qb, kb], 0).reshape(2 * HID, 1).astype(np.float32)
    vwt = np.ascontiguousarray(np.asarray(vw, np.float32).T).astype(ml_dtypes.bfloat16)  # [256,256]

    inv = 1.0 / np.sqrt(np.asarray(bn_var, np.float64) + BN_EPS)
    A = (gamma * np.asarray(bn_w, np.float64) * inv).astype(np.float32)  # [256]
    Bp = (gamma * ((np.asarray(conv_b, np.float64) - np.asarray(bn_mean, np.float64))
                   * np.asarray(bn_w, np.float64) * inv + np.asarray(bn_b, np.float64))).astype(np.float32)
    Wp = (np.asarray(conv_w, np.float32) * A[:, None, None, None]).astype(np.float32)  # [o,i,dy,dx]
    # layout [icc, ic128, tap, occ, oc128] -> [2, 128, 9*2*128]
    cwt = np.ascontiguousarray(
        Wp.transpose(1, 2, 3, 0).reshape(2, 128, 3, 3, 2, 128).reshape(2, 128, 9 * 2 * 128)
    ).astype(ml_dtypes.bfloat16)

    gnlvb = (gamma_nl * np.asarray(vb, np.float32)).reshape(C, 1).astype(np.float32)
    bprime = Bp.reshape(C, 1)
    gnlrow = np.full((1, 128), gamma_nl, ml_dtypes.bfloat16)
    ones128 = np.ones((128, 1), ml_dtypes.bfloat16)

    in_maps = []
    for core in range(N_CORES):
        r = core % 4
        m = np.zeros(12, np.float32)
        m[0] = 1.0 if r == 2 else 0.0   # top halo from G0.bottom
        m[1] = 1.0 if r == 3 else 0.0   # top halo from G1.bottom
        m[2] = 1.0 if r == 0 else 0.0   # bottom halo from G2.top
        m[3] = 1.0 if r == 1 else 0.0   # bottom halo from G3.top
        m[4] = 1.0 if r == 1 else 0.0   # left halo from G0.right
        m[5] = 1.0 if r == 3 else 0.0   # left halo from G2.right
        m[6] = 1.0 if r == 0 else 0.0   # right halo from G1.left
        m[7] = 1.0 if r == 2 else 0.0   # right halo from G3.left
        m[8] = 1.0 if r == 3 else 0.0   # TL corner from g_top0[:,63]
        m[9] = 1.0 if r == 2 else 0.0   # TR corner from g_top1[:,0]
        m[10] = 1.0 if r == 1 else 0.0  # BL corner from g_bot0[:,63]
        m[11] = 1.0 if r == 0 else 0.0  # BR corner from g_bot1[:,0]
        hmask = np.tile(m[None, :], (128, 1)).astype(np.float32)
        in_maps.append({
            "x": np.ascontiguousarray(blocks[core]),
            "qkwt": qkwt, "qkb": qkb, "vwt": vwt, "cwt": cwt,
            "gnlvb": gnlvb, "bprime": bprime, "hmask": hmask,
            "gnlrow": gnlrow, "ones128": ones128,
        })
    return in_maps


def _assemble(outs):
    ob = np.stack([np.asarray(o, np.float32) for o in outs], 0)  # [8, 256, 4096]
    return (
        ob.reshape(B, S, S, C, HB, WB)
        .transpose(0, 3, 1, 4, 2, 5)
        .reshape(B, C, H, W)
    )


def run_on_hw(in_maps, trace=False, trace_kwargs=None):
    from concourse import bass_utils
    nc = _get_nc()
    res = bass_utils.run_bass_kernel_spmd(
        nc, in_maps, core_ids=list(range(N_CORES)), trace=trace,
        **(trace_kwargs or {}))
    return res


def kernel(**inputs):
    in_maps = _prep_inputs(**inputs)
    res = run_on_hw(in_maps)
    outs = [r["out"] for r in res.results]
    return _assemble(outs)


if __name__ == "__main__":
    pass
